# revision 17
# baseline (speedup 1.0000x reference)
"""Trainium2 Bass kernel for AdaptiveSemiseparableLayer (v2).

Reference computation (B=4, L=2048, D=R=2048, DH=512):
    t_out = depthwise_conv1d(x, conv_w, k=3) + conv_b
    u = x @ Wu.T + bu ; v = x @ Wv.T + bv
    gates = sigmoid(relu(x @ Wg1.T + bg1) @ Wg2.T + bg2)
    cs = cumsum(u * gates, axis=seq)
    out = t_out + (cs * (v * gates)) @ Wu.T

Sharding: sequence-parallel; 8192 tokens -> 8 contiguous 1024-token blocks
(one per core; each block lies inside one batch row). The only cross-core
dependency is the cumsum carry: odd core c needs core c-1's column sums.
That is a pairwise AllReduce(add) over (R,) = 8KB; each core then forms
carry = (pair_sum - own_sum) * is_odd on GPSIMD.

v2 changes vs the original kernel (637 us):
  - two-level cumsum: 8 independent 128-wide triangular matmuls per r-tile
    (8 MMs, 1024 PE cycles) instead of the 12-MM trapezoid; the 8-block
    prefix is recombined per 128-token window on GPSIMD via a scan +
    scalar_tensor_tensor (cs + P) * vg.
  - block sums come from the intra-cumsum PSUM's last columns (DVE strided
    gather + reduce) -- the 128 free-dim-1 block-sum matmuls are gone.
  - the carry selection matmuls (mask MMs) are gone (pairwise AllReduce).
  - all bias matmuls are gone: bg1/bg2(GT) ride the ACT activation bias,
    bv rides a scalar_tensor_tensor, bu/bg2(G-natural) have a DVE
    broadcast-add fallback only when nonzero (they are zero here).
  - carry/got chain runs on GPSIMD so the ~70us collective latency never
    blocks the DVE/PE pipelines.
  - Wg1/Wg2 preloaded whole; x loaded in 16 per-k chunks for startup
    overlap; gn/intra and ug/gos share SBUF pools (phase-disjoint).

Layouts (per core, T=1024 local tokens):
    xT    [D, T+2]  bf16   transposed shard with conv halo columns
    h     [dh-part, t]     (4 tiles)
    gn    [t-part, r]      (8 tiles)   natural gates (for ug)
    ug    [t-part, r]      bf16, lhsT of the intra cumsum matmuls
    intra [r-part, t]      bf16, block-local inclusive cumsum
    gtt/vgt [r-part, t]    transposed gates / gated v
    got   [r-part, t]      (cs_local + prefix + carry) * vg
    outT  [d-part, t]      final, conv fused in epilogue
"""

import numpy as np
import ml_dtypes
from contextlib import ExitStack

import concourse.bass as bass
import concourse.mybir as mybir
import concourse.tile as tile
from concourse.bass_utils import run_bass_kernel_spmd

P = 128
B, L, D = 4, 2048, 2048
R, DH = 2048, 512
NCORES = 8
T = (B * L) // NCORES          # 1024 tokens per core
TH = T + 2                     # with conv halo
ND, NR, NDH, NT = D // P, R // P, DH // P, T // P
TC = 512                       # matmul free-dim chunk (one PSUM bank of f32)
NTC = T // TC                  # 2
NRC = R // TC                  # 4
BF = mybir.dt.bfloat16
F32 = mybir.dt.float32
AF = mybir.ActivationFunctionType
ALU = mybir.AluOpType

import os
DG = int(os.environ.get("K_DG", "8"))     # got lag behind gtv emission


def _emit(nc, io, zu, zg2):
    """zu/zg2: True when bu / bg2 are all-zero (skip broadcast adds)."""
    ctx = ExitStack()
    tc = io["tc"]
    pool = lambda name, bufs, **kw: ctx.enter_context(
        tc.tile_pool(name=name, bufs=bufs, **kw)
    )
    const = pool("const", 1)
    xpool = pool("xpool", 1)
    hpool = pool("hpool", NDH)
    gnp = pool("gnp", NT)          # gn tiles; reused for intra pairs
    ugp = pool("ugp", 2 * NT)      # ug tiles [128,1024]; reused for gos
    pbp = pool("pbp", NR)          # Pb [128, 9] f32 per rk
    ptp = pool("ptp", 3)           # Ptot transient
    gtp = pool("gtp", 2)
    vgp = pool("vgp", DG + 2)
    wutp = pool("wutp", 6)
    wvtp = pool("wvtp", 8)
    wu2p = pool("wu2p", 5)
    outp = pool("outp", 2)
    ctp = pool("ctp", 4)
    psum = pool("psum", 8, space="PSUM")
    dram = pool("dram", 1, space="DRAM")

    xT, WuT, WvT, Wg1T, Wg2T = (io[k] for k in ["xT", "WuT", "WvT", "Wg1T", "Wg2T"])
    outT = io["outT"]

    # ---- critical-path loads first: Wg1 (4 chunks) + x shard (16 chunks)
    bg1c = const.tile([P, NDH], F32)
    nc.scalar.dma_start(out=bg1c, in_=io["bg1_col"][:].rearrange("(k p) -> p k", p=P))
    # x chunks on the sync queue, Wg1 chunks interleaved on the scalar queue:
    # the two queues pull in parallel, and phase H consumes chunk k at
    # ~1.7us/chunk, above the per-queue arrival rate after the first chunks.
    wg1 = const.tile([P, ND, DH], BF)
    xtile = xpool.tile([P, ND, TH], BF)
    for g in range(4):
        nc.scalar.dma_start(
            out=wg1[:, g * 4 : (g + 1) * 4, :],
            in_=Wg1T[g * 4 * P : (g + 1) * 4 * P, :].rearrange(
                "(k p) c -> p k c", p=P
            ),
        )
        for k in range(4 * g, 4 * g + 4):
            nc.sync.dma_start(
                out=xtile[:, k, :], in_=xT[k * P : (k + 1) * P, :]
            )
    xs = [xtile[:, kd, :] for kd in range(ND)]

    # ---- remaining constants (needed from phase G on), scalar queue
    wg2 = const.tile([P, NDH, R], BF)
    nc.scalar.dma_start(
        out=wg2, in_=Wg2T[:, :].rearrange("(k p) c -> p k c", p=P)
    )
    tri = const.tile([P, P], BF)
    nc.scalar.dma_start(out=tri, in_=io["tri"][:, :])
    bg2c = const.tile([P, NR], F32)
    nc.scalar.dma_start(out=bg2c, in_=io["bg2_col"][:].rearrange("(k p) -> p k", p=P))
    bvc = const.tile([P, NR], F32)
    nc.scalar.dma_start(out=bvc, in_=io["bv_col"][:].rearrange("(k p) -> p k", p=P))
    cw = const.tile([P, ND, 3], F32)
    nc.scalar.dma_start(out=cw, in_=io["conv_w2"][:, :].rearrange("(k p) c -> p k c", p=P))
    cb = const.tile([P, ND], F32)
    nc.scalar.dma_start(out=cb, in_=io["conv_b2"][:].rearrange("(k p) -> p k", p=P))
    oddc = const.tile([P, 1], F32)
    nc.scalar.dma_start(out=oddc, in_=io["odd"][:].rearrange("(p k) -> p k", k=1))
    ones8 = const.tile([P, NT], F32)
    nc.vector.memset(ones8, 1.0)
    if not zu:
        buB = const.tile([P, R], BF)
        nc.scalar.dma_start(out=buB, in_=io["buB"][:, :])
    if not zg2:
        bg2B = const.tile([P, R], BF)
        nc.scalar.dma_start(out=bg2B, in_=io["bg2B"][:, :])

    # ---- phase H: hs[kdh] [dh-part, t] = relu(Wg1 @ x^T + bg1)
    # k-outer with all 8 PSUM banks: each x chunk is consumed once, as it
    # arrives, so the PE is paced by DMA for at most the first chunk or two.
    hs = [hpool.tile([P, T], BF, name=f"h{kdh}", tag="h") for kdh in range(NDH)]
    psh = [[psum.tile([P, TC], F32, name=f"psh{kdh}{c}", tag="ps")
            for c in range(NTC)] for kdh in range(NDH)]
    for k in range(ND):
        for kdh in range(NDH):
            for c in range(NTC):
                nc.tensor.matmul(
                    psh[kdh][c],
                    lhsT=wg1[:, k, kdh * P : (kdh + 1) * P],
                    rhs=xs[k][:, 1 + c * TC : 1 + (c + 1) * TC],
                    start=(k == 0),
                    stop=(k == ND - 1),
                )
    for kdh in range(NDH):
        for c in range(NTC):
            # relu(psh + bg1) on DVE: the 8 evacuations all land at the end
            # of the k-outer H loop, and DVE drains them 2x faster than ACT
            # so phase G's first matmuls are gated ~1us, not ~5us.
            nc.vector.tensor_scalar(
                hs[kdh][:, c * TC : (c + 1) * TC],
                psh[kdh][c],
                bg1c[:, kdh : kdh + 1],
                0.0,
                op0=ALU.add,
                op1=ALU.max,
            )

    # ---- phase G: gn[t] [t-part, r] = sigmoid(h @ Wg2^T (+ bg2))
    gn = [gnp.tile([P, R], BF, name=f"gn{t}", tag="gn") for t in range(NT)]
    for rc in range(NRC):
        for h2 in range(2):
            psg = [psum.tile([P, TC], F32, name=f"psg{tt}", tag="ps")
                   for tt in range(NT // 2)]
            for kdh in range(NDH):
                for tt in range(NT // 2):
                    t = h2 * (NT // 2) + tt
                    nc.tensor.matmul(
                        psg[tt],
                        lhsT=hs[kdh][:, t * P : (t + 1) * P],
                        rhs=wg2[:, kdh, rc * TC : (rc + 1) * TC],
                        start=(kdh == 0),
                        stop=(kdh == NDH - 1),
                    )
            for tt in range(NT // 2):
                t = h2 * (NT // 2) + tt
                if not zg2:
                    nc.vector.tensor_add(
                        out=psg[tt], in0=psg[tt],
                        in1=bg2B[:, rc * TC : (rc + 1) * TC],
                    )
                nc.scalar.activation(
                    out=gn[t][:, rc * TC : (rc + 1) * TC], in_=psg[tt],
                    func=AF.Sigmoid,
                )

    # ---- phase U: ug[t] = (x @ Wu^T (+ bu)) * gn   [t-part, r]
    # ug tiles are [128, 1024]: two 512-wide rc chunks per tile.
    ugt = [[ugp.tile([P, 2 * TC], BF, name=f"ug_{t}_{rp}", tag="ug")
            for rp in range(2)] for t in range(NT)]
    bs_dram = dram.tile([R], F32)
    for rc in range(NRC):
        psu = [psum.tile([P, TC], F32, name=f"psu{t}", tag="ps") for t in range(NT)]
        for k in range(ND):
            wut = wutp.tile([P, TC], BF, name="wut", tag="wut")
            nc.sync.dma_start(
                out=wut, in_=WuT[k * P : (k + 1) * P, rc * TC : (rc + 1) * TC]
            )
            for t in range(NT):
                nc.tensor.matmul(
                    psu[t],
                    lhsT=xs[k][:, 1 + t * P : 1 + t * P + P],
                    rhs=wut,
                    start=(k == 0),
                    stop=(k == ND - 1),
                )
        for t in range(NT):
            dst = ugt[t][rc // 2][:, (rc % 2) * TC : (rc % 2 + 1) * TC]
            if not zu:
                nc.vector.tensor_add(
                    out=psu[t], in0=psu[t],
                    in1=buB[:, rc * TC : (rc + 1) * TC],
                )
            nc.vector.tensor_mul(
                out=dst, in0=psu[t], in1=gn[t][:, rc * TC : (rc + 1) * TC],
            )

    # ---- phase CS: block-local cumsum per r-tile; extract block sums
    # intra[rk] lives in the gn pool (pairs): intra tile i holds rk=2i, 2i+1.
    intra_t = [gnp.tile([P, R], BF, name=f"in{i}", tag="gn") for i in range(NT)]
    intra = [intra_t[rk // 2][:, (rk % 2) * T : (rk % 2 + 1) * T] for rk in range(NR)]
    bs_sb = const.tile([P, NR], F32)
    pbs = []
    deferred = []

    def flush_copy():
        frk, fps = deferred.pop(0)
        nc.vector.tensor_copy(out=intra[frk][:, 0:TC], in_=fps[0])
        nc.vector.tensor_copy(out=intra[frk][:, TC:T], in_=fps[1])

    for rk in range(NR):
        rc4, ri4 = rk // 4, rk % 4
        ugsl = lambda j: ugt[j][rc4 // 2][:, (rc4 % 2) * TC + ri4 * P :
                                          (rc4 % 2) * TC + (ri4 + 1) * P]
        pscs = [psum.tile([P, TC], F32, name=f"pscs{c}", tag="ps") for c in range(2)]
        for j in range(NT):
            nc.tensor.matmul(
                pscs[j // 4][:, (j % 4) * P : (j % 4 + 1) * P],
                lhsT=ugsl(j),
                rhs=tri,
                start=True,
                stop=True,
            )
        # Pb gathers + block-sum reduce FIRST (they gate the collective);
        # the fat intra evacuation copies are deferred 2 r-tiles so the
        # bs chain completes ~10us earlier on the DVE.
        pb = pbp.tile([P, NT + 1], F32, name=f"pb{rk}", tag="pb")
        nc.vector.memset(pb[:, 0:1], 0.0)
        nc.vector.tensor_copy(out=pb[:, 1:5], in_=pscs[0][:, P - 1 :: P])
        nc.vector.tensor_copy(out=pb[:, 5:9], in_=pscs[1][:, P - 1 :: P])
        nc.vector.tensor_reduce(
            out=bs_sb[:, rk : rk + 1], in_=pb[:, 1:9],
            axis=mybir.AxisListType.X, op=ALU.add,
        )
        pbs.append(pb)
        deferred.append((rk, pscs))
        if len(deferred) > 2:
            flush_copy()
    while deferred:
        flush_copy()

    # ---- pairwise AllReduce of block sums
    pr_dram = dram.tile([R], F32)
    nc.sync.dma_start(out=bs_dram[:].rearrange("(k p) -> p k", p=P), in_=bs_sb)
    nc.gpsimd.collective_compute(
        "AllReduce",
        ALU.add,
        replica_groups=[[2 * i, 2 * i + 1] for i in range(NCORES // 2)],
        ins=[bs_dram[:].opt()],
        outs=[pr_dram[:].opt()],
    )
    prs = const.tile([P, NR], F32)
    nc.gpsimd.dma_start(out=prs, in_=pr_dram[:].rearrange("(k p) -> p k", p=P))
    carry = const.tile([P, NR], F32)

    # ---- phase GT/V (PE+ACT+DVE) with got recombination on GPSIMD
    vgts = [None] * NR
    gos = [ugp.tile([P, T], BF, name=f"go{rk}", tag="ug") for rk in range(NR)]

    def emit_gtv(rk):
        gtt = gtp.tile([P, T], BF, name="gtt", tag="gtt")
        psgt = [psum.tile([P, TC], F32, name=f"psgt{c}", tag="ps") for c in range(NTC)]
        for kdh in range(NDH):
            for c in range(NTC):
                nc.tensor.matmul(
                    psgt[c],
                    lhsT=wg2[:, kdh, rk * P : (rk + 1) * P],
                    rhs=hs[kdh][:, c * TC : (c + 1) * TC],
                    start=(kdh == 0),
                    stop=(kdh == NDH - 1),
                )
        for c in range(NTC):
            nc.scalar.activation(
                out=gtt[:, c * TC : (c + 1) * TC],
                in_=psgt[c],
                func=AF.Sigmoid,
                bias=bg2c[:, rk : rk + 1],
            )
        vgt = vgp.tile([P, T], BF, name="vgt", tag="vgt")
        psv = [psum.tile([P, TC], F32, name=f"psv{c}", tag="ps") for c in range(NTC)]
        for kg in range(ND // 4):
            wvt = wvtp.tile([P, 4, P], BF, name="wvt", tag="wvt")
            nc.sync.dma_start(
                out=wvt,
                in_=WvT[kg * 4 * P : (kg + 1) * 4 * P,
                        rk * P : (rk + 1) * P].rearrange("(g p) c -> p g c", p=P),
            )
            for i in range(4):
                k = kg * 4 + i
                for c in range(NTC):
                    nc.tensor.matmul(
                        psv[c],
                        lhsT=wvt[:, i, :],
                        rhs=xs[k][:, 1 + c * TC : 1 + (c + 1) * TC],
                        start=(k == 0),
                        stop=(k == ND - 1),
                    )
        for c in range(NTC):
            nc.vector.scalar_tensor_tensor(
                out=vgt[:, c * TC : (c + 1) * TC],
                in0=psv[c],
                scalar=bvc[:, rk : rk + 1],
                in1=gtt[:, c * TC : (c + 1) * TC],
                op0=ALU.add,
                op1=ALU.mult,
            )
        return vgt

    GATE = int(os.environ.get("K_GATE", "6"))

    def emit_got(rk):
        if rk == 0:
            # The tile scheduler reorders by dependency, not emission order,
            # so a bare carry chain would be scheduled as soon as the DVE has
            # a gap -- stalling the whole DVE FIFO until the AllReduce lands.
            # This dummy 1-element copy makes the carry tile depend on
            # vgt(GATE), pinning the chain behind GATE r-tiles of GT/V work.
            nc.vector.tensor_copy(out=carry[0:1, 0:1], in_=vgts[GATE][0:1, 0:1])
            nc.vector.tensor_sub(out=carry, in0=prs, in1=bs_sb)
            nc.vector.tensor_scalar_mul(carry, carry, oddc[:, 0:1])
        # P_tot[:, j] = carry + sum_{j'<j} bsum_j'
        pt = ptp.tile([P, NT], F32, name="pt", tag="pt")
        nc.vector.tensor_tensor_scan(
            out=pt,
            data0=ones8,
            data1=pbs[rk][:, 0:NT],
            initial=carry[:, rk : rk + 1],
            op0=ALU.mult,
            op1=ALU.add,
        )
        for j in range(NT):
            nc.vector.scalar_tensor_tensor(
                out=gos[rk][:, j * P : (j + 1) * P],
                in0=intra[rk][:, j * P : (j + 1) * P],
                scalar=pt[:, j : j + 1],
                in1=vgts[rk][:, j * P : (j + 1) * P],
                op0=ALU.add,
                op1=ALU.mult,
            )
        vgts[rk] = None

    for i in range(NR + DG):
        if i < NR:
            vgts[i] = emit_gtv(i)
        j = i - DG
        if j >= 0:
            emit_got(j)

    # ---- phase UV + conv epilogue: outT[d, t] = got-proj + conv + conv_b
    # The conv term ct depends only on x, so it is emitted BEFORE the kd's
    # matmuls: the DVE computes it while the PE accumulates, and the last
    # tile's epilogue is just one add + DMA after the final matmul.
    for kd in range(ND):
        wu2 = []
        for rg in range(NR // 4):
            wu24 = wu2p.tile([P, 4, P], BF, name="wu24", tag="wu24")
            nc.sync.dma_start(
                out=wu24,
                in_=WuT[rg * 4 * P : (rg + 1) * 4 * P,
                        kd * P : (kd + 1) * P].rearrange("(g p) c -> p g c", p=P),
            )
            wu2.append(wu24)
        cts = []
        for c in range(NTC):
            ct = ctp.tile([P, TC], F32, name="ct", tag="ct")
            # dummy dep: ct chains depend only on x, and the scheduler would
            # otherwise run all 32 of them right after x lands -- ahead of
            # the phase-H relu evacuations, starving phase G. Gating each on
            # gos[kd] pins them into the UV phase where the DVE is idle.
            nc.vector.tensor_copy(out=ct[0:1, 0:1], in_=gos[kd][0:1, 0:1])
            nc.vector.tensor_scalar(
                ct,
                xs[kd][:, c * TC : c * TC + TC],
                cw[:, kd, 0:1],
                cb[:, kd : kd + 1],
                op0=ALU.mult,
                op1=ALU.add,
            )
            nc.vector.scalar_tensor_tensor(
                out=ct,
                in0=xs[kd][:, c * TC + 1 : c * TC + 1 + TC],
                scalar=cw[:, kd, 1:2],
                in1=ct,
                op0=ALU.mult,
                op1=ALU.add,
            )
            nc.vector.scalar_tensor_tensor(
                out=ct,
                in0=xs[kd][:, c * TC + 2 : c * TC + 2 + TC],
                scalar=cw[:, kd, 2:3],
                in1=ct,
                op0=ALU.mult,
                op1=ALU.add,
            )
            cts.append(ct)
        psuv = [psum.tile([P, TC], F32, name=f"psuv{c}", tag="ps") for c in range(NTC)]
        for rk in range(NR):
            for c in range(NTC):
                nc.tensor.matmul(
                    psuv[c],
                    lhsT=wu2[rk // 4][:, rk % 4, :],
                    rhs=gos[rk][:, c * TC : (c + 1) * TC],
                    start=(rk == 0),
                    stop=(rk == NR - 1),
                )
        for c in range(NTC):
            ob = outp.tile([P, TC], F32, name="ob", tag="ob")
            nc.vector.tensor_add(out=ob, in0=psuv[c], in1=cts[c])
            if kd >= ND - 2:
                # split the drain of the last tiles across four queues: the
                # final out DMA is the kernel's critical tail.
                for qi, eng in enumerate((nc.sync, nc.gpsimd, nc.scalar, nc.gpsimd)):
                    eng.dma_start(
                        out=outT[kd * P + qi * 32 : kd * P + (qi + 1) * 32,
                                 c * TC : (c + 1) * TC],
                        in_=ob[qi * 32 : (qi + 1) * 32, :],
                    )
            else:
                nc.sync.dma_start(
                    out=outT[kd * P : (kd + 1) * P, c * TC : (c + 1) * TC], in_=ob
                )
    ctx.close()


def _split_multi_waits(nc):
    """The walrus build in this env allows only ONE attached sync-wait per
    instruction; hoist extra waits onto standalone InstEventSemaphore ops
    inserted just before, on the same engine (semantically identical)."""
    import bass_rust

    n = 0
    for blk in nc.m.functions[0].blocks:
        changed = False
        out = []
        for ins in blk.instructions:
            si = getattr(ins, "sync_info", None)
            if si is not None and len(si.on_wait) > 1:
                waits = list(si.on_wait)
                for w in waits[:-1]:
                    ev = mybir.InstEventSemaphore(name=f"WSPLIT-{n}", ins=[], outs=[])
                    n += 1
                    ev.engine = ins.engine
                    ev.sync_info = bass_rust.SyncInfo(on_wait=[w], on_update=[])
                    out.append(ev)
                ins.sync_info = bass_rust.SyncInfo(
                    on_wait=[waits[-1]], on_update=list(si.on_update)
                )
                changed = True
            out.append(ins)
        if changed:
            try:
                blk.instructions[:] = out
            except TypeError:
                blk.instructions = out
    return n


def _build(zu, zg2):
    nc = bass.Bass(num_devices=NCORES)
    io = {}
    io["xT"] = nc.declare_dram_parameter("xT", [D, TH], BF, False)
    io["WuT"] = nc.declare_dram_parameter("WuT", [D, R], BF, False)
    io["WvT"] = nc.declare_dram_parameter("WvT", [D, R], BF, False)
    io["Wg1T"] = nc.declare_dram_parameter("Wg1T", [D, DH], BF, False)
    io["Wg2T"] = nc.declare_dram_parameter("Wg2T", [DH, R], BF, False)
    io["tri"] = nc.declare_dram_parameter("tri", [P, P], BF, False)
    io["bg1_col"] = nc.declare_dram_parameter("bg1_col", [DH], F32, False)
    io["bg2_col"] = nc.declare_dram_parameter("bg2_col", [R], F32, False)
    io["bv_col"] = nc.declare_dram_parameter("bv_col", [R], F32, False)
    io["conv_w2"] = nc.declare_dram_parameter("conv_w2", [D, 3], F32, False)
    io["conv_b2"] = nc.declare_dram_parameter("conv_b2", [D], F32, False)
    io["odd"] = nc.declare_dram_parameter("odd", [P], F32, False)
    if not zu:
        io["buB"] = nc.declare_dram_parameter("buB", [P, R], BF, False)
    if not zg2:
        io["bg2B"] = nc.declare_dram_parameter("bg2B", [P, R], BF, False)
    io["outT"] = nc.declare_dram_parameter("outT", [D, T], F32, True)
    with tile.TileContext(nc, num_cores=NCORES) as tc:
        io["tc"] = tc
        _emit(nc, io, zu, zg2)
    _split_multi_waits(nc)
    return nc


_NC_CACHE = {}


def _get_nc(zu, zg2):
    key = (zu, zg2)
    if key not in _NC_CACHE:
        _NC_CACHE[key] = _build(zu, zg2)
    return _NC_CACHE[key]


def _prep_in_maps(x, Wu, bu, Wv, bv, Wg1, bg1, Wg2, bg2, conv_w, conv_b):
    bf = ml_dtypes.bfloat16
    f32 = np.float32
    x = np.asarray(x, f32)
    bu = np.asarray(bu, f32)
    bg2 = np.asarray(bg2, f32)
    zu = not bu.any()
    zg2 = not bg2.any()
    shared = dict(
        WuT=np.asarray(Wu, f32).T.astype(bf),
        WvT=np.asarray(Wv, f32).T.astype(bf),
        Wg1T=np.asarray(Wg1, f32).T.astype(bf),
        Wg2T=np.asarray(Wg2, f32).T.astype(bf),
        tri=(np.arange(P)[:, None] <= np.arange(P)[None, :]).astype(bf),
        bg1_col=np.ascontiguousarray(np.asarray(bg1, f32)),
        bg2_col=np.ascontiguousarray(bg2),
        bv_col=np.ascontiguousarray(np.asarray(bv, f32)),
        conv_w2=np.ascontiguousarray(np.asarray(conv_w, f32)[:, 0, :]),
        conv_b2=np.ascontiguousarray(np.asarray(conv_b, f32)),
    )
    if not zu:
        shared["buB"] = np.broadcast_to(bu.astype(bf), (P, R)).copy()
    if not zg2:
        shared["bg2B"] = np.broadcast_to(bg2.astype(bf), (P, R)).copy()
    xflat = x.reshape(B * L, D)
    in_maps = []
    for c in range(NCORES):
        xh = np.zeros((TH, D), f32)
        xh[1 : T + 1] = xflat[c * T : (c + 1) * T]
        if c % 2 == 1:
            xh[0] = xflat[c * T - 1]
        else:
            xh[T + 1] = xflat[(c + 1) * T]
        odd = np.full((P,), float(c % 2), f32)
        in_maps.append(dict(shared, xT=xh.T.astype(bf), odd=odd))
    return in_maps, zu, zg2


def _assemble(results):
    out = np.empty((B * L, D), np.float32)
    for c in range(NCORES):
        out[c * T : (c + 1) * T] = np.asarray(results[c]["outT"]).T
    return out.reshape(B, L, D)


def kernel(x, Wu, bu, Wv, bv, Wg1, bg1, Wg2, bg2, conv_w, conv_b):
    in_maps, zu, zg2 = _prep_in_maps(
        x, Wu, bu, Wv, bv, Wg1, bg1, Wg2, bg2, conv_w, conv_b
    )
    res = run_bass_kernel_spmd(
        _get_nc(zu, zg2), in_maps, core_ids=list(range(NCORES))
    )
    return _assemble(res.results)


def run_traced(inputs):
    """Profiled run: returns (output, exec_time_ns)."""
    in_maps, zu, zg2 = _prep_in_maps(**inputs)
    res = run_bass_kernel_spmd(
        _get_nc(zu, zg2), in_maps, core_ids=list(range(NCORES)), trace=True
    )
    return _assemble(res.results), res.exec_time_ns
